# revision 21
# baseline (speedup 1.0000x reference)
"""Trainium2 Bass kernel for nn_BiomechanicsLoss (masked quadratic-form loss).

Math (per point): et = [u0, v1, w2, .5(u1+v0), .5(u2+w0), .5(w1+v2)],
q = et^T C et with C = inv(compliance) cast to f32.  Loss =
sqrt(sum_masked(q^2)) / count_masked, mask = gt_sdf < 1e-8.

q = et^T sym(C) et; sym(C) is block-diagonal: 3x3 normal block A over
s1,s2,s3 = u0,v1,w2 plus d = 0.25*Cs[3,3] times (s4^2+s5^2+s6^2) with
s4 = u1+v0, s5 = u2+w0, s6 = w1+v2.  The constants satisfy w11 == w22 and
w13 == w23, so polarization gives a pure sum of 7 squares:
  q = P'^2 + b12*(s1^2+s2^2) + b3*s3^2 + d*(s4^2+s5^2+s6^2)
  P' = (a1/sqrt2)(s1+s2) + (a3/sqrt2) s3,  a1 = sqrt(w12), a3 = w13/a1,
  b12 = w11 - w12/2, b3 = w33 - a3^2/2   (all positive).

Engine split (per 512-point-per-partition chunk, software-pipelined):
  DVE:   mask via tensor_scalar(is_lt) with fused count accum, p12' wide
         scaled copy (tensor_scalar 4x; a1 == a2 so one op covers s1,s2),
         s456 = ONE wide add (host packs [u1 u2 w1 | v0 w0 v2]),
         P' via scalar_tensor_tensor, qm = q*mask read from PSUM
  Pool:  tP = p1'+p2', zP = P'^2 (plain tensor_tensor only)
  ScalarE: the 3 weighted Square activations (wide: zb12 2F, zb3 F,
         z456 3F) and the final Square(qm) with accum_out ->
         per-partition sum(mask*q^2)
  PE:    THE FOLD -- 7 identity-weight matmuls accumulate the 7 square
         terms into one PSUM bank (q per point), freeing DVE of the
         fold-adds entirely.
Host packs bf16 (halves HBM traffic; ~27us DMA at ~340GB/s/core) and the
host sums the [128, 2*NT] per-core partials, sqrts, divides.
"""

import numpy as np

N = 4_194_304
NCORES = 8
N_LOCAL = N // NCORES  # 524288
P = 128
J = N_LOCAL // P  # 4096 points per partition (partition-major layout)
F = 512  # chunk width == one PSUM bank of f32
CHUNKS = [F] * (J // F)  # 8 chunks
NT = len(CHUNKS)

THRESH = 1e-8


def _weights():
    vp, Ep = 0.4, 0.21
    Ci = np.zeros((6, 6), dtype=np.float64)
    Ci[0, 0] = 1 / Ep;  Ci[0, 1] = -vp / Ep; Ci[0, 2] = -vp / Ep
    Ci[1, 0] = -vp / Ep; Ci[1, 1] = 1 / Ep;  Ci[1, 2] = -vp / Ep
    Ci[2, 0] = -vp;      Ci[2, 1] = -vp;     Ci[2, 2] = 1 / Ep
    Ci[3, 3] = 2 * (1 + vp) / Ep
    Ci[4, 4] = Ci[3, 3]
    Ci[5, 5] = Ci[3, 3]
    # match reference: inverse computed in f64, cast to f32
    C = np.linalg.inv(Ci).astype(np.float32).astype(np.float64)
    Cs = 0.5 * (C + C.T)
    A = Cs[:3, :3]
    d = 0.25 * Cs[3, 3]
    w11, w33 = A[0, 0], A[2, 2]
    w12, w13, w23 = 2 * A[0, 1], 2 * A[0, 2], 2 * A[1, 2]
    assert abs(A[0, 0] - A[1, 1]) < 1e-12 and abs(w13 - w23) < 1e-12
    a1 = np.sqrt(w12)
    a3 = w13 / a1
    b12 = w11 - w12 / 2
    b3 = w33 - a3 * a3 / 2
    assert b12 > 0 and b3 > 0
    return dict(
        a12s=float(a1 / np.sqrt(2)), a3s=float(a3 / np.sqrt(2)),
        rb12=float(np.sqrt(b12)), c3=float(b3 / b12),
        rd=float(np.sqrt(d)),
    )


_NC = None


def _build_nc():
    import concourse.bacc as bacc
    import concourse.mybir as mybir
    import concourse.tile as tile
    from concourse import masks

    W = _weights()

    f32 = mybir.dt.float32
    bf16 = mybir.dt.bfloat16
    Sq = mybir.ActivationFunctionType.Square
    ALU = mybir.AluOpType

    nc = bacc.Bacc()
    packed = nc.dram_tensor("packed", [P, 10 * J], bf16, kind="ExternalInput")
    out = nc.dram_tensor("out", [P, 2 * NT], f32, kind="ExternalOutput")

    with tile.TileContext(nc) as tc:
        with (
            tc.tile_pool(name="singles", bufs=1) as singles,
            tc.tile_pool(name="io", bufs=4) as io,
            tc.tile_pool(name="mid", bufs=3) as mid,
            tc.tile_pool(name="ps", bufs=2, space="PSUM") as ps,
            tc.tile_pool(name="stats", bufs=1) as stats_pool,
        ):
            ident = singles.tile([P, P], bf16)
            masks.make_identity(nc, ident[:])
            # c3-scaled identity: lets the PE fold apply the b3/b12 ratio to
            # the s3 square so ScalarE squares all of [s1 s2 s3] in ONE op
            identc = singles.tile([P, P], bf16)
            nc.scalar.mul(identc, ident, W["c3"])
            stats = stats_pool.tile([P, 2 * NT], f32)

            actx = {}
            pctx = {}

            def stage_a(t):
                buf = io.tile([P, 10 * F], bf16, tag="buf")
                nc.sync.dma_start(
                    out=buf[:], in_=packed[:, t * 10 * F:(t + 1) * 10 * F])
                s12 = buf[:, 6 * F:8 * F]
                s3 = buf[:, 8 * F:9 * F]
                sd = buf[:, 9 * F:10 * F]

                m = mid.tile([P, F], bf16, tag="m")
                nc.vector.tensor_scalar(
                    out=m, in0=sd, scalar1=THRESH, scalar2=None, op0=ALU.is_lt,
                    op1=ALU.add, accum_out=stats[:, NT + t:NT + t + 1])
                p12 = mid.tile([P, 2 * F], bf16, tag="p12")
                nc.vector.tensor_scalar_mul(p12, s12, W["a12s"])
                s456 = mid.tile([P, 3 * F], bf16, tag="s456")
                nc.vector.tensor_add(s456, buf[:, 0:3 * F], buf[:, 3 * F:6 * F])

                tP = mid.tile([P, F], bf16, tag="tP")
                nc.gpsimd.tensor_add(tP, p12[:, 0:F], p12[:, F:2 * F])
                Pv = mid.tile([P, F], bf16, tag="Pv")
                nc.vector.scalar_tensor_tensor(
                    Pv, s3, W["a3s"], tP, ALU.mult, ALU.add)
                zP = mid.tile([P, F], bf16, tag="zP")
                nc.gpsimd.tensor_mul(zP, Pv, Pv)

                # weighted squares on ScalarE, two wide ops:
                # X = [zb1 zb2 zb3' | z4 z5 z6], zb3' carries scale rb12 and
                # the PE fold corrects it by c3 = b3/b12
                X = mid.tile([P, 6 * F], bf16, tag="X")
                nc.scalar.activation(X[:, 0:3 * F], buf[:, 6 * F:9 * F], Sq,
                                     scale=W["rb12"])
                nc.scalar.activation(X[:, 3 * F:6 * F], s456, Sq, scale=W["rd"])
                actx[t] = (m, X, zP)

            def stage_pe(t):
                m, X, zP = actx.pop(t)
                # fold the 7 square terms into one PSUM bank via
                # (scaled-)identity-weight accumulating matmuls; the s3
                # block (k == 2) picks up the b3/b12 correction
                qp = ps.tile([P, F], f32)
                for k in range(6):
                    wgt = identc if k == 2 else ident
                    nc.tensor.matmul(qp[:], wgt[:], X[:, k * F:(k + 1) * F],
                                     start=(k == 0), stop=False)
                nc.tensor.matmul(qp[:], ident[:], zP[:], start=False, stop=True)
                pctx[t] = (m, qp)

            def stage_b(t):
                m, qp = pctx.pop(t)
                qm = mid.tile([P, F], bf16, tag="qm")
                nc.vector.tensor_mul(qm, qp, m)
                junk = mid.tile([P, F], bf16, tag="junk")
                nc.scalar.activation(
                    junk, qm, Sq, accum_out=stats[:, t:t + 1])

            # software pipeline, 3 chunks in flight:
            # A(0) A(1) [A(2) PE(0) B(0)] [A(3) PE(1) B(1)] ...
            stage_a(0)
            stage_a(1)
            for t in range(NT):
                if t + 2 < NT:
                    stage_a(t + 2)
                stage_pe(t)
                stage_b(t)

            nc.sync.dma_start(out=out[:, :], in_=stats[:])

    nc.compile()
    return nc


def _get_nc():
    global _NC
    if _NC is None:
        _NC = _build_nc()
    return _NC


def _run(in_maps, trace=False, **kwargs):
    from concourse.bass_utils import run_bass_kernel_spmd

    nc = _get_nc()
    return run_bass_kernel_spmd(
        nc, in_maps, core_ids=list(range(NCORES)), trace=trace, **kwargs)


def _make_in_maps(grad_u, grad_v, grad_w, gt_sdf):
    import ml_dtypes

    bf = ml_dtypes.bfloat16
    grad_u = np.asarray(grad_u, dtype=np.float32).astype(bf)
    grad_v = np.asarray(grad_v, dtype=np.float32).astype(bf)
    grad_w = np.asarray(grad_w, dtype=np.float32).astype(bf)
    gt_sdf = np.asarray(gt_sdf, dtype=np.float32).astype(bf)
    in_maps = []
    for c in range(NCORES):
        sl = slice(c * N_LOCAL, (c + 1) * N_LOCAL)
        gu = grad_u[sl].reshape(P, J, 3)
        gv = grad_v[sl].reshape(P, J, 3)
        gw = grad_w[sl].reshape(P, J, 3)
        sd = gt_sdf[sl].reshape(P, J)
        parts = []
        off = 0
        for Fc in CHUNKS:
            s = slice(off, off + Fc)
            parts += [gu[:, s, 1], gu[:, s, 2], gw[:, s, 1],   # u1 u2 w1
                      gv[:, s, 0], gw[:, s, 0], gv[:, s, 2],   # v0 w0 v2
                      gu[:, s, 0], gv[:, s, 1],                # s1 s2
                      gw[:, s, 2],                             # s3
                      sd[:, s]]
            off += Fc
        packed = np.ascontiguousarray(np.concatenate(parts, axis=1))
        in_maps.append({"packed": packed})
    return in_maps


def _finalize(results):
    ssq = 0.0
    cnt = 0.0
    for res in results:
        st = np.asarray(res["out"], dtype=np.float64)
        ssq += st[:, :NT].sum()
        cnt += st[:, NT:].sum()
    Wv = np.sqrt(ssq)
    return np.float32(Wv / cnt)


def kernel(grad_u, grad_v, grad_w, gt_sdf):
    in_maps = _make_in_maps(grad_u, grad_v, grad_w, gt_sdf)
    res = _run(in_maps, trace=False)
    return _finalize(res.results)
